# revision 17
# baseline (speedup 1.0000x reference)
"""Trainium2 Bass kernel for nn_BaselinePhasorBlock (B=2, L=1024, D=512, K=64).

Algorithm restructure: the phasor-memory cumsum
    retrieved[t,d] = Re[ sum_k e^{-i q[t,k]} * sum_{s<=t} e^{i key[s,k]} v[s,d] ]
collapses to causal attention:
    A[t,s] = cosQ[t]·cosK[s] + sinQ[t]·sinK[s]   (dot over k)
    retrieved = tril(A) @ value
so nothing of size (L,K,D) is ever materialized.

LayerNorm folding (exact):
    LN(retrieved/norm) @ Wo + bo + x
  = scale_t * (r @ Wg - mu_t * cw) + [x + ln_b@Wo + bo]
with Wg = diag(ln_g)@Wo, cw = colsums(Wg), scale_t = 1/sqrt(var_r + eps*norm_t^2),
norm_t^2 = (t+1)*K.  (LN row stats are scale-invariant up to the eps term, which
is handled exactly by folding norm^2 into the eps.)

Sharding (8 cores, SPMD, no collectives): core c -> batch b = c//4, strip pair
i = c%4 owning t-strips [i*128, (i+1)*128) and [(7-i)*128, (8-i)*128).  The
pairing makes causal work uniform; causality is enforced by a per-core mask on
the score matrix (AT layout [s, t]).  Each core computes its batch's keys and
values over the full sequence (redundant across the 4 cores of a batch, but
avoids collectives entirely).

All matmuls run in bf16 (validated: 1.8e-3 scale-relative error end-to-end),
fp32 PSUM accumulation, fp32 residual/output path.
"""

import math
from contextlib import ExitStack

import numpy as np

B, L, D, K = 2, 1024, 512, 64
PI = math.pi
NCORES = 8
SPB = 128  # strip size (rows per t-strip); 2 strips per core
NSC = L // 128  # number of 128-row s-chunks (8)
NDC = D // 128  # number of 128-wide d chunks (4)
EPS = 1e-5

_CACHE = {}


def _build_program(gelu_override=None):
    import concourse.bass as bass
    import concourse.bacc as bacc
    import concourse.mybir as mybir
    import concourse.tile as tile

    AF = mybir.ActivationFunctionType
    GELU = AF.Gelu if gelu_override is None else gelu_override
    FP32 = mybir.dt.float32
    BF16 = mybir.dt.bfloat16

    nc = bacc.Bacc()

    # ---- per-core DRAM parameters (names must match in_maps keys) ----
    d_xT = nc.declare_dram_parameter("xT", [4, 128, L], BF16, isOutput=False)
    d_qxT = nc.declare_dram_parameter("qxT", [4, 128, 256], BF16, isOutput=False)
    d_wk1 = nc.declare_dram_parameter("wk1", [4, 128, D], BF16, isOutput=False)
    d_wq1 = nc.declare_dram_parameter("wq1", [4, 128, D], BF16, isOutput=False)
    d_wv = nc.declare_dram_parameter("wv", [4, 128, D], BF16, isOutput=False)
    d_wg = nc.declare_dram_parameter("wg", [4, 128, D], BF16, isOutput=False)
    d_wk2d = nc.declare_dram_parameter("wk2d", [4, 128, 128], BF16, isOutput=False)
    d_wq2d = nc.declare_dram_parameter("wq2d", [4, 128, 128], BF16, isOutput=False)
    d_bk1 = nc.declare_dram_parameter("bk1", [4, 128, 1], FP32, isOutput=False)
    d_bq1 = nc.declare_dram_parameter("bq1", [4, 128, 1], FP32, isOutput=False)
    d_bk2d = nc.declare_dram_parameter("bk2d", [128, 1], FP32, isOutput=False)
    d_bq2d = nc.declare_dram_parameter("bq2d", [128, 1], FP32, isOutput=False)
    d_bv = nc.declare_dram_parameter("bv", [1, D], FP32, isOutput=False)
    d_cw = nc.declare_dram_parameter("cw", [1, D], BF16, isOutput=False)
    d_mask = nc.declare_dram_parameter("mask", [NSC, 128, 256], BF16, isOutput=False)
    d_xplus = nc.declare_dram_parameter("xplus", [2, 128, D], FP32, isOutput=False)
    d_epsn2 = nc.declare_dram_parameter("epsn2", [2, 128, 1], FP32, isOutput=False)
    d_out = nc.declare_dram_parameter("out", [2, 128, D], FP32, isOutput=True)

    with tile.TileContext(nc) as tc, ExitStack() as ctx:
        consts = ctx.enter_context(tc.tile_pool(name="consts", bufs=1))
        work = ctx.enter_context(tc.tile_pool(name="work", bufs=1))
        atm_pool = ctx.enter_context(tc.tile_pool(name="atm", bufs=3))
        small = ctx.enter_context(tc.tile_pool(name="small", bufs=1))
        ps_big = ctx.enter_context(tc.tile_pool(name="ps_big", bufs=2, space="PSUM"))
        ps_at = ctx.enter_context(tc.tile_pool(name="ps_at", bufs=2, space="PSUM"))
        ps_rt = ctx.enter_context(tc.tile_pool(name="ps_rt", bufs=1, space="PSUM"))
        ps_stat = ctx.enter_context(tc.tile_pool(name="ps_stat", bufs=1, space="PSUM"))

        # ---- load constants/weights into SBUF ----
        xT = consts.tile([128, 4, L], BF16)
        qxT = consts.tile([128, 4, 256], BF16)
        wk1 = consts.tile([128, 4, D], BF16)
        wq1 = consts.tile([128, 4, D], BF16)
        wv = consts.tile([128, 4, D], BF16)
        wg = consts.tile([128, 4, D], BF16)
        wk2d = consts.tile([128, 4, 128], BF16)
        wq2d = consts.tile([128, 4, 128], BF16)
        bk1 = consts.tile([128, 4, 1], FP32)
        bq1 = consts.tile([128, 4, 1], FP32)
        bk2d = consts.tile([128, 1], FP32)
        bq2d = consts.tile([128, 1], FP32)
        bvb = consts.tile([128, D], FP32)
        cw = consts.tile([1, D], BF16)
        maskt = consts.tile([128, NSC, 256], BF16)
        xplus = consts.tile([128, 2, D], FP32)
        epsn2 = consts.tile([128, 2, 1], FP32)
        ones = consts.tile([128, 1], BF16)
        cosbias = consts.tile([128, 1], FP32)
        sinscale = consts.tile([128, 1], FP32)

        for c in range(4):
            nc.sync.dma_start(out=wk1[:, c, :], in_=d_wk1[c])
            nc.sync.dma_start(out=wq1[:, c, :], in_=d_wq1[c])
            nc.sync.dma_start(out=xT[:, c, :], in_=d_xT[c])
            nc.sync.dma_start(out=qxT[:, c, :], in_=d_qxT[c])
            nc.sync.dma_start(out=wv[:, c, :], in_=d_wv[c])
            nc.sync.dma_start(out=wg[:, c, :], in_=d_wg[c])
            nc.sync.dma_start(out=wk2d[:, c, :], in_=d_wk2d[c])
            nc.sync.dma_start(out=wq2d[:, c, :], in_=d_wq2d[c])
            nc.sync.dma_start(out=bk1[:, c, :], in_=d_bk1[c])
            nc.sync.dma_start(out=bq1[:, c, :], in_=d_bq1[c])
        nc.sync.dma_start(out=bk2d, in_=d_bk2d[:])
        nc.sync.dma_start(out=bq2d, in_=d_bq2d[:])
        nc.sync.dma_start(out=bvb, in_=d_bv[:].to_broadcast((128, D)))
        nc.sync.dma_start(out=cw, in_=d_cw[:])
        for sc in range(NSC):
            nc.sync.dma_start(out=maskt[:, sc, :], in_=d_mask[sc])
        for st in range(2):
            nc.sync.dma_start(out=xplus[:, st, :], in_=d_xplus[st])
            nc.sync.dma_start(out=epsn2[:, st, :], in_=d_epsn2[st])
        nc.vector.memset(ones, 1.0)
        nc.vector.memset(cosbias[0:64, :], PI / 2)
        nc.vector.memset(cosbias[64:128, :], 0.0)
        nc.vector.memset(sinscale[0:64, :], -PI)
        nc.vector.memset(sinscale[64:128, :], PI)

        # ---- working SBUF tiles ----
        hkT = work.tile([128, 4, L], BF16)     # gelu(x@Wk1+b) transposed
        hqT = work.tile([128, 4, 256], BF16)
        kph2 = work.tile([128, L], BF16)       # tanh phase, duplicated halves
        qph2 = work.tile([128, 256], BF16)
        KS = work.tile([128, L], BF16)         # rows 0:64 cosK, 64:128 sinK
        QS = work.tile([128, 256], BF16)
        value = work.tile([128, NSC, D], BF16)  # value rows [s,d] per s-chunk
        rT_sb = work.tile([128, 4, 256], BF16)  # retrievedT [d, t]
        rsq = work.tile([128, 4, 256], BF16)
        out_sb = work.tile([128, 2, D], FP32)

        # ---- MLP1 (key): hkT[j, m] = gelu(Wk1^T @ xT + bk1) ----
        for m in range(2):  # m-chunks of 512 sequence positions
            for j in range(4):  # dout chunks
                ps = ps_big.tile([128, 512], FP32, tag="mlp")
                for c in range(4):  # din chunks
                    nc.tensor.matmul(
                        ps,
                        lhsT=wk1[:, c, j * 128:(j + 1) * 128],
                        rhs=xT[:, c, m * 512:(m + 1) * 512],
                        start=(c == 0),
                        stop=(c == 3),
                    )
                nc.scalar.activation(
                    out=hkT[:, j, m * 512:(m + 1) * 512], in_=ps,
                    func=GELU, bias=bk1[:, j, :], scale=1.0,
                )

        # ---- MLP1 (query) on the core's 256 query columns ----
        for j in range(4):
            ps = ps_big.tile([128, 512], FP32, tag="mlp")
            for c in range(4):
                nc.tensor.matmul(
                    ps[:, :256],
                    lhsT=wq1[:, c, j * 128:(j + 1) * 128],
                    rhs=qxT[:, c, :],
                    start=(c == 0),
                    stop=(c == 3),
                )
            nc.scalar.activation(
                out=hqT[:, j, :], in_=ps[:, :256],
                func=GELU, bias=bq1[:, j, :], scale=1.0,
            )

        # ---- phase matmuls + tanh (key) ----
        # wk2d doubles Wk2 so rows 0:64 and 64:128 both get the phase.
        for m in range(2):
            ps = ps_big.tile([128, 512], FP32, tag="mlp")
            for j in range(4):
                nc.tensor.matmul(
                    ps,
                    lhsT=wk2d[:, j, :],
                    rhs=hkT[:, j, m * 512:(m + 1) * 512],
                    start=(j == 0),
                    stop=(j == 3),
                )
            nc.scalar.activation(
                out=kph2[:, m * 512:(m + 1) * 512], in_=ps,
                func=AF.Tanh, bias=bk2d, scale=1.0,
            )
        # ---- phase matmuls + tanh (query) ----
        ps = ps_big.tile([128, 512], FP32, tag="mlp")
        for j in range(4):
            nc.tensor.matmul(
                ps[:, :256],
                lhsT=wq2d[:, j, :],
                rhs=hqT[:, j, :],
                start=(j == 0),
                stop=(j == 3),
            )
        nc.scalar.activation(
            out=qph2, in_=ps[:, :256], func=AF.Tanh, bias=bq2d, scale=1.0,
        )

        # ---- value rows: value[s, d] = x@Wv + bv ----
        for sc in range(NSC):
            ps = ps_big.tile([128, 512], FP32, tag="mlp")
            for c in range(4):
                nc.tensor.matmul(
                    ps,
                    lhsT=xT[:, c, sc * 128:(sc + 1) * 128],
                    rhs=wv[:, c, :],
                    start=(c == 0),
                    stop=(c == 3),
                )
            nc.vector.tensor_add(out=value[:, sc, :], in0=ps, in1=bvb)

        # ---- cos/sin of phases (stacked halves: 0:64 cos, 64:128 sin) ----
        # ACT Sin only accepts [-pi, pi].  With t = tanh(phase/pi) in (-1,1):
        #   cos(pi*t) = sin(pi/2 - pi*|t|)   (parity; arg stays in range)
        #   sin(pi*t) = sin(pi*t)
        # so take |t| on the cos half, then one Sin pass with per-partition
        # scale (-pi top / +pi bottom) and bias (pi/2 top / 0 bottom).
        nc.scalar.activation(out=kph2[0:64, :], in_=kph2[0:64, :], func=AF.Abs)
        nc.scalar.activation(out=qph2[0:64, :], in_=qph2[0:64, :], func=AF.Abs)
        nc.scalar.activation(out=KS, in_=kph2, func=AF.Sin,
                             bias=cosbias, scale=sinscale)
        nc.scalar.activation(out=QS, in_=qph2, func=AF.Sin,
                             bias=cosbias, scale=sinscale)

        # ---- scores + causal mask + retrievedT accumulation ----
        rt_ps = ps_rt.tile([128, 4, 256], FP32)
        for sc in range(NSC):
            at_ps = ps_at.tile([128, 256], FP32, tag="at")
            nc.tensor.matmul(
                at_ps,
                lhsT=KS[:, sc * 128:(sc + 1) * 128],
                rhs=QS,
                start=True,
                stop=True,
            )
            atm = atm_pool.tile([128, 256], BF16, tag="atm")
            nc.vector.tensor_mul(out=atm, in0=at_ps, in1=maskt[:, sc, :])
            # PSUM has_written clears are per-BANK on start: rt_ps spans 2
            # banks (dc 0,1 | dc 2,3), so exactly one start per bank (first
            # matmul in it) and one stop on each bank's last matmul.
            for dc in range(NDC):
                nc.tensor.matmul(
                    rt_ps[:, dc, :],
                    lhsT=value[:, sc, dc * 128:(dc + 1) * 128],
                    rhs=atm,
                    start=(sc == 0 and dc in (0, 2)),
                    stop=(sc == NSC - 1 and dc in (1, 3)),
                )

        # ---- copy retrievedT to SBUF + squares ----
        for dc in range(NDC):
            nc.vector.tensor_copy(rT_sb[:, dc, :], rt_ps[:, dc, :])
        for dc in range(NDC):
            nc.scalar.activation(out=rsq[:, dc, :], in_=rt_ps[:, dc, :],
                                 func=AF.Square, bias=0.0, scale=1.0)

        # ---- row stats: sums / sumsq in [t,1]; row-sums in [1,t] ----
        # All 16 column-wise stat matmuls share one PSUM bank -> one
        # accumulation group: single start (clears the bank), single stop.
        sums_ps = ps_stat.tile([128, 4], FP32)  # cols: [sum s0, sum s1, sq s0, sq s1]
        row_ps = ps_stat.tile([1, 256], FP32, tag="row")
        first = True
        for st in range(2):
            for src, col in ((rT_sb, st), (rsq, 2 + st)):
                for dc in range(NDC):
                    nc.tensor.matmul(
                        sums_ps[:, col:col + 1],
                        lhsT=src[:, dc, st * 128:(st + 1) * 128],
                        rhs=ones,
                        start=first,
                        stop=(st == 1 and col == 3 and dc == 3),
                    )
                    first = False
        for dc in range(NDC):
            nc.tensor.matmul(
                row_ps,
                lhsT=ones,
                rhs=rT_sb[:, dc, :],
                start=(dc == 0),
                stop=(dc == 3),
            )

        # negmu_row = -(row sums)/D  (bf16, feeds the rank-1 mean-fold matmul)
        negmu = small.tile([1, 256], BF16)
        nc.vector.tensor_scalar_mul(out=negmu, in0=row_ps, scalar1=-1.0 / D)

        # per-strip scale_t = 1/sqrt(var + eps*norm^2)
        import concourse.mybir as _mb
        ALU = _mb.AluOpType
        mu = small.tile([128, 2], FP32)
        musq = small.tile([128, 2], FP32)
        var = small.tile([128, 2], FP32)
        scl = small.tile([128, 2], FP32)
        for st in range(2):
            nc.vector.tensor_scalar_mul(out=mu[:, st:st + 1],
                                        in0=sums_ps[:, st:st + 1], scalar1=1.0 / D)
            nc.vector.tensor_mul(out=musq[:, st:st + 1],
                                 in0=mu[:, st:st + 1], in1=mu[:, st:st + 1])
            # var = sumsq/D - mu^2
            nc.vector.scalar_tensor_tensor(
                out=var[:, st:st + 1],
                in0=sums_ps[:, 2 + st:3 + st],
                scalar=1.0 / D,
                in1=musq[:, st:st + 1],
                op0=ALU.mult,
                op1=ALU.subtract,
            )
        for st in range(2):
            nc.scalar.activation(out=scl[:, st:st + 1], in_=var[:, st:st + 1],
                                 func=AF.Sqrt, bias=epsn2[:, st, :], scale=1.0)
            nc.vector.reciprocal(out=scl[:, st:st + 1], in_=scl[:, st:st + 1])

        # ---- output: out = scale * (rT^T @ Wg - mu*cw) + xplus ----
        for st in range(2):
            ps = ps_big.tile([128, 512], FP32, tag="mlp")
            for dc in range(NDC):
                nc.tensor.matmul(
                    ps,
                    lhsT=rT_sb[:, dc, st * 128:(st + 1) * 128],
                    rhs=wg[:, dc, :],
                    start=(dc == 0),
                    stop=False,
                )
            nc.tensor.matmul(
                ps,
                lhsT=negmu[:, st * 128:(st + 1) * 128],
                rhs=cw,
                start=False,
                stop=True,
            )
            nc.vector.scalar_tensor_tensor(
                out=out_sb[:, st, :],
                in0=ps,
                scalar=scl[:, st:st + 1],
                in1=xplus[:, st, :],
                op0=ALU.mult,
                op1=ALU.add,
            )
            nc.sync.dma_start(out=d_out[st], in_=out_sb[:, st, :])

    return nc


def _host_prepare(inputs):
    """Build the 8 per-core input maps (all host-side numpy)."""
    import ml_dtypes

    bf16 = ml_dtypes.bfloat16
    f32 = np.float32

    x = np.asarray(inputs["x"], f32)
    Wk1 = np.asarray(inputs["Wk1"], f32)
    bk1 = np.asarray(inputs["bk1"], f32)
    Wk2 = np.asarray(inputs["Wk2"], f32)
    bk2 = np.asarray(inputs["bk2"], f32)
    Wq1 = np.asarray(inputs["Wq1"], f32)
    bq1 = np.asarray(inputs["bq1"], f32)
    Wq2 = np.asarray(inputs["Wq2"], f32)
    bq2 = np.asarray(inputs["bq2"], f32)
    Wv = np.asarray(inputs["Wv"], f32)
    bv = np.asarray(inputs["bv"], f32)
    ln_g = np.asarray(inputs["ln_g"], f32)
    ln_b = np.asarray(inputs["ln_b"], f32)
    Wo = np.asarray(inputs["Wo"], f32)
    bo = np.asarray(inputs["bo"], f32)

    Wg = (ln_g[:, None] * Wo).astype(bf16)
    cw = Wg.astype(f32).sum(axis=0).astype(bf16).reshape(1, D)
    out_bias = (ln_b @ Wo + bo).astype(f32)

    def chunks(w):  # [D, F] -> [4, 128, F] bf16
        return np.ascontiguousarray(
            w.reshape(4, 128, -1).astype(bf16))

    wk2d = np.concatenate([Wk2, Wk2], axis=1)  # [512, 128]
    wq2d = np.concatenate([Wq2, Wq2], axis=1)

    base = {
        "wk1": chunks(Wk1),
        "wq1": chunks(Wq1),
        "wv": chunks(Wv),
        "wg": np.ascontiguousarray(Wg.reshape(4, 128, D)),
        "wk2d": chunks(wk2d),
        "wq2d": chunks(wq2d),
        "bk1": np.ascontiguousarray(bk1.reshape(4, 128, 1)),
        "bq1": np.ascontiguousarray(bq1.reshape(4, 128, 1)),
        "bk2d": np.concatenate([bk2, bk2]).reshape(128, 1).astype(f32),
        "bq2d": np.concatenate([bq2, bq2]).reshape(128, 1).astype(f32),
        "bv": bv.reshape(1, D).astype(f32),
        "cw": cw,
    }

    in_maps = []
    for c in range(NCORES):
        b, i = divmod(c, 4)
        t0, t1 = i * 128, (7 - i) * 128
        xb = x[b]  # [L, D]
        xTb = np.ascontiguousarray(xb.T)  # [D, L]
        qx = np.concatenate([xTb[:, t0:t0 + 128], xTb[:, t1:t1 + 128]], axis=1)
        tglob = np.concatenate([np.arange(t0, t0 + 128), np.arange(t1, t1 + 128)])
        srange = np.arange(L)
        mask = (srange[:, None] <= tglob[None, :]).astype(bf16)  # [L, 256]
        xplus = np.stack([xb[t0:t0 + 128], xb[t1:t1 + 128]]) + out_bias
        epsn2 = (EPS * K * (tglob.astype(f32) + 1.0)).reshape(2, 128, 1)
        m = dict(base)
        m["xT"] = np.ascontiguousarray(xTb.reshape(4, 128, L).astype(bf16))
        m["qxT"] = np.ascontiguousarray(qx.reshape(4, 128, 256).astype(bf16))
        m["mask"] = np.ascontiguousarray(mask.reshape(NSC, 128, 256))
        m["xplus"] = np.ascontiguousarray(xplus.astype(f32))
        m["epsn2"] = np.ascontiguousarray(epsn2.astype(f32))
        in_maps.append(m)
    return in_maps


def run(inputs, trace=False):
    from concourse.bass_utils import run_bass_kernel_spmd

    if "nc" not in _CACHE:
        nc = _build_program()
        nc.finalize()
        _CACHE["nc"] = nc
    nc = _CACHE["nc"]
    in_maps = _host_prepare(inputs)
    res = run_bass_kernel_spmd(nc, in_maps, list(range(NCORES)), trace=trace)
    out = np.empty((B, L, D), np.float32)
    for c in range(NCORES):
        b, i = divmod(c, 4)
        oc = np.asarray(res.results[c]["out"], np.float32)
        out[b, i * 128:(i + 1) * 128] = oc[0]
        out[b, (7 - i) * 128:(8 - i) * 128] = oc[1]
    return out, res


def kernel(**inputs):
    out, _ = run(inputs, trace=False)
    return out


# revision 21
# speedup vs baseline: 1.2704x; 1.2704x over previous
"""Trainium2 Bass kernel for nn_BaselinePhasorBlock (B=2, L=1024, D=512, K=64).

Algorithm restructure: the phasor-memory cumsum
    retrieved[t,d] = Re[ sum_k e^{-i q[t,k]} * sum_{s<=t} e^{i key[s,k]} v[s,d] ]
collapses to causal attention:
    A[t,s] = cosQ[t]·cosK[s] + sinQ[t]·sinK[s]   (dot over k)
    retrieved = tril(A) @ value
so nothing of size (L,K,D) is ever materialized.

LayerNorm folding (exact):
    LN(retrieved/norm) @ Wo + bo + x
  = scale_t * (r @ Wg - mu_t * cw) + [x + ln_b@Wo + bo]
with Wg = diag(ln_g)@Wo, cw = colsums(Wg), scale_t = 1/sqrt(var_r + eps*norm_t^2),
norm_t^2 = (t+1)*K.  (LN row stats are scale-invariant up to the eps term, which
is handled exactly by folding norm^2 into the eps.)

Sharding (8 cores, SPMD, no collectives): core c -> batch b = c//4, strip pair
i = c%4 owning t-strips [i*128, (i+1)*128) and [(7-i)*128, (8-i)*128).  The
pairing makes causal work uniform; causality is enforced by a per-core mask on
the score matrix (AT layout [s, t]).  Each core computes its batch's keys and
values over the full sequence (redundant across the 4 cores of a batch, but
avoids collectives entirely).

All matmuls run in bf16 (validated: 1.8e-3 scale-relative error end-to-end),
fp32 PSUM accumulation, fp32 residual/output path.
"""

import math
from contextlib import ExitStack

import numpy as np

B, L, D, K = 2, 1024, 512, 64
PI = math.pi
NCORES = 8
SPB = 128  # strip size (rows per t-strip); 2 strips per core
NSC = L // 128  # number of 128-row s-chunks (8)
NDC = D // 128  # number of 128-wide d chunks (4)
EPS = 1e-5

_CACHE = {}


def _build_program(gelu_override=None):
    import concourse.bass as bass
    import concourse.bacc as bacc
    import concourse.mybir as mybir
    import concourse.tile as tile

    AF = mybir.ActivationFunctionType
    GELU = AF.Gelu if gelu_override is None else gelu_override
    FP32 = mybir.dt.float32
    BF16 = mybir.dt.bfloat16

    nc = bacc.Bacc()

    # ---- per-core DRAM parameters (names must match in_maps keys) ----
    # All host-packed to the exact SBUF tile layout: one DMA per tensor.
    d_wk1 = nc.declare_dram_parameter("wk1", [128, 4, D], BF16, isOutput=False)
    d_xTa = nc.declare_dram_parameter("xTa", [128, 4, 512], BF16, isOutput=False)
    d_bk1 = nc.declare_dram_parameter("bk1", [128, 4], FP32, isOutput=False)
    d_bk2d = nc.declare_dram_parameter("bk2d", [128, 1], FP32, isOutput=False)
    d_wk2d = nc.declare_dram_parameter("wk2d", [128, 4, 128], BF16, isOutput=False)
    d_xTb = nc.declare_dram_parameter("xTb", [128, 4, 512], BF16, isOutput=False)
    d_wq1 = nc.declare_dram_parameter("wq1", [128, 4, D], BF16, isOutput=False)
    d_qxT = nc.declare_dram_parameter("qxT", [128, 4, 256], BF16, isOutput=False)
    d_bq1 = nc.declare_dram_parameter("bq1", [128, 4], FP32, isOutput=False)
    d_bq2d = nc.declare_dram_parameter("bq2d", [128, 1], FP32, isOutput=False)
    d_wq2d = nc.declare_dram_parameter("wq2d", [128, 4, 128], BF16, isOutput=False)
    d_wv = nc.declare_dram_parameter("wv", [128, 4, D], BF16, isOutput=False)
    d_bv = nc.declare_dram_parameter("bv", [1, D], FP32, isOutput=False)
    d_cw = nc.declare_dram_parameter("cw", [1, D], BF16, isOutput=False)
    d_mask = nc.declare_dram_parameter("mask", [128, NSC, 256], BF16, isOutput=False)
    d_wg = nc.declare_dram_parameter("wg", [128, 4, D], BF16, isOutput=False)
    d_xplus = nc.declare_dram_parameter("xplus", [128, 2, D], FP32, isOutput=False)
    d_epsn2 = nc.declare_dram_parameter("epsn2", [128, 2], FP32, isOutput=False)
    d_out = nc.declare_dram_parameter("out", [2, 128, D], FP32, isOutput=True)

    with tile.TileContext(nc) as tc, ExitStack() as ctx:
        consts = ctx.enter_context(tc.tile_pool(name="consts", bufs=1))
        work = ctx.enter_context(tc.tile_pool(name="work", bufs=1))
        atm_pool = ctx.enter_context(tc.tile_pool(name="atm", bufs=3))
        small = ctx.enter_context(tc.tile_pool(name="small", bufs=1))
        ps_big = ctx.enter_context(tc.tile_pool(name="ps_big", bufs=2, space="PSUM"))
        ps_at = ctx.enter_context(tc.tile_pool(name="ps_at", bufs=2, space="PSUM"))
        ps_rt = ctx.enter_context(tc.tile_pool(name="ps_rt", bufs=1, space="PSUM"))
        ps_stat = ctx.enter_context(tc.tile_pool(name="ps_stat", bufs=1, space="PSUM"))

        # ---- load constants/weights into SBUF ----
        xT = consts.tile([128, 4, L], BF16)
        qxT = consts.tile([128, 4, 256], BF16)
        wk1 = consts.tile([128, 4, D], BF16)
        wq1 = consts.tile([128, 4, D], BF16)
        wv = consts.tile([128, 4, D], BF16)
        wg = consts.tile([128, 4, D], BF16)
        wk2d = consts.tile([128, 4, 128], BF16)
        wq2d = consts.tile([128, 4, 128], BF16)
        bk1 = consts.tile([128, 4], FP32)
        bq1 = consts.tile([128, 4], FP32)
        bk2d = consts.tile([128, 1], FP32)
        bq2d = consts.tile([128, 1], FP32)
        bvb = consts.tile([128, D], FP32)
        cw = consts.tile([1, D], BF16)
        maskt = consts.tile([128, NSC, 256], BF16)
        xplus = consts.tile([128, 2, D], FP32)
        epsn2 = consts.tile([128, 2], FP32)
        ones = consts.tile([128, 1], BF16)
        cosbias = consts.tile([128, 1], FP32)
        sinscale = consts.tile([128, 1], FP32)

        # Issue order == need order; critical path on sync (HWDGE), the
        # late-phase constants on gpsimd's queue.
        nc.sync.dma_start(out=wk1, in_=d_wk1[:])
        nc.sync.dma_start(out=xT[:, :, 0:512], in_=d_xTa[:])
        nc.sync.dma_start(out=bk1, in_=d_bk1[:])
        nc.sync.dma_start(out=bk2d, in_=d_bk2d[:])
        nc.sync.dma_start(out=wk2d, in_=d_wk2d[:])
        nc.sync.dma_start(out=xT[:, :, 512:1024], in_=d_xTb[:])
        nc.sync.dma_start(out=wq1, in_=d_wq1[:])
        nc.sync.dma_start(out=qxT, in_=d_qxT[:])
        nc.sync.dma_start(out=bq1, in_=d_bq1[:])
        nc.sync.dma_start(out=bq2d, in_=d_bq2d[:])
        nc.sync.dma_start(out=wq2d, in_=d_wq2d[:])
        nc.sync.dma_start(out=wv, in_=d_wv[:])
        nc.sync.dma_start(out=bvb, in_=d_bv[:].to_broadcast((128, D)))
        nc.gpsimd.dma_start(out=maskt, in_=d_mask[:])
        nc.gpsimd.dma_start(out=wg, in_=d_wg[:])
        nc.gpsimd.dma_start(out=cw, in_=d_cw[:])
        nc.gpsimd.dma_start(out=xplus, in_=d_xplus[:])
        nc.gpsimd.dma_start(out=epsn2, in_=d_epsn2[:])
        nc.vector.memset(ones, 1.0)
        nc.vector.memset(cosbias[0:64, :], PI / 2)
        nc.vector.memset(cosbias[64:128, :], 0.0)
        nc.vector.memset(sinscale[0:64, :], -PI)
        nc.vector.memset(sinscale[64:128, :], PI)

        # ---- working SBUF tiles ----
        hkT = work.tile([128, 4, L], BF16)     # gelu(x@Wk1+b) transposed
        hqT = work.tile([128, 4, 256], BF16)
        kph2 = work.tile([128, L], BF16)       # tanh phase, duplicated halves
        qph2 = work.tile([128, 256], BF16)
        KS = work.tile([128, L], BF16)         # rows 0:64 cosK, 64:128 sinK
        QS = work.tile([128, 256], BF16)
        value = work.tile([128, NSC, D], BF16)  # value rows [s,d] per s-chunk
        rT_sb = work.tile([128, 4, 256], BF16)  # retrievedT [d, t]
        rsq = work.tile([128, 4, 256], BF16)
        out_sb = work.tile([128, 2, D], FP32)

        # ---- MLP1 (key): hkT[j, m] = gelu(Wk1^T @ xT + bk1) ----
        for m in range(2):  # m-chunks of 512 sequence positions
            for j in range(4):  # dout chunks
                ps = ps_big.tile([128, 512], FP32, tag="mlp")
                for c in range(4):  # din chunks
                    nc.tensor.matmul(
                        ps,
                        lhsT=wk1[:, c, j * 128:(j + 1) * 128],
                        rhs=xT[:, c, m * 512:(m + 1) * 512],
                        start=(c == 0),
                        stop=(c == 3),
                    )
                nc.scalar.activation(
                    out=hkT[:, j, m * 512:(m + 1) * 512], in_=ps,
                    func=GELU, bias=bk1[:, j:j + 1], scale=1.0,
                )

        # ---- MLP1 (query) on the core's 256 query columns ----
        for j in range(4):
            ps = ps_big.tile([128, 512], FP32, tag="mlp")
            for c in range(4):
                nc.tensor.matmul(
                    ps[:, :256],
                    lhsT=wq1[:, c, j * 128:(j + 1) * 128],
                    rhs=qxT[:, c, :],
                    start=(c == 0),
                    stop=(c == 3),
                )
            nc.scalar.activation(
                out=hqT[:, j, :], in_=ps[:, :256],
                func=GELU, bias=bq1[:, j:j + 1], scale=1.0,
            )

        # ---- phase matmuls + tanh (key) ----
        # wk2d doubles Wk2 so rows 0:64 and 64:128 both get the phase.
        for m in range(2):
            ps = ps_big.tile([128, 512], FP32, tag="mlp")
            for j in range(4):
                nc.tensor.matmul(
                    ps,
                    lhsT=wk2d[:, j, :],
                    rhs=hkT[:, j, m * 512:(m + 1) * 512],
                    start=(j == 0),
                    stop=(j == 3),
                )
            nc.scalar.activation(
                out=kph2[:, m * 512:(m + 1) * 512], in_=ps,
                func=AF.Tanh, bias=bk2d, scale=1.0,
            )
        # ---- phase matmuls + tanh (query) ----
        ps = ps_big.tile([128, 512], FP32, tag="mlp")
        for j in range(4):
            nc.tensor.matmul(
                ps[:, :256],
                lhsT=wq2d[:, j, :],
                rhs=hqT[:, j, :],
                start=(j == 0),
                stop=(j == 3),
            )
        nc.scalar.activation(
            out=qph2, in_=ps[:, :256], func=AF.Tanh, bias=bq2d, scale=1.0,
        )

        # ---- value rows: value[s, d] = x@Wv + bv ----
        for sc in range(NSC):
            ps = ps_big.tile([128, 512], FP32, tag="mlp")
            for c in range(4):
                nc.tensor.matmul(
                    ps,
                    lhsT=xT[:, c, sc * 128:(sc + 1) * 128],
                    rhs=wv[:, c, :],
                    start=(c == 0),
                    stop=(c == 3),
                )
            nc.vector.tensor_add(out=value[:, sc, :], in0=ps, in1=bvb)

        # ---- cos/sin of phases (stacked halves: 0:64 cos, 64:128 sin) ----
        # ACT Sin only accepts [-pi, pi].  With t = tanh(phase/pi) in (-1,1):
        #   cos(pi*t) = sin(pi/2 - pi*|t|)   (parity; arg stays in range)
        #   sin(pi*t) = sin(pi*t)
        # so take |t| on the cos half, then one Sin pass with per-partition
        # scale (-pi top / +pi bottom) and bias (pi/2 top / 0 bottom).
        nc.scalar.activation(out=kph2[0:64, :], in_=kph2[0:64, :], func=AF.Abs)
        nc.scalar.activation(out=qph2[0:64, :], in_=qph2[0:64, :], func=AF.Abs)
        nc.scalar.activation(out=KS, in_=kph2, func=AF.Sin,
                             bias=cosbias, scale=sinscale)
        nc.scalar.activation(out=QS, in_=qph2, func=AF.Sin,
                             bias=cosbias, scale=sinscale)

        # ---- scores + causal mask + retrievedT accumulation ----
        rt_ps = ps_rt.tile([128, 4, 256], FP32)
        for sc in range(NSC):
            at_ps = ps_at.tile([128, 256], FP32, tag="at")
            nc.tensor.matmul(
                at_ps,
                lhsT=KS[:, sc * 128:(sc + 1) * 128],
                rhs=QS,
                start=True,
                stop=True,
            )
            atm = atm_pool.tile([128, 256], BF16, tag="atm")
            nc.vector.tensor_mul(out=atm, in0=at_ps, in1=maskt[:, sc, :])
            # PSUM has_written clears are per-BANK on start: rt_ps spans 2
            # banks (dc 0,1 | dc 2,3), so exactly one start per bank (first
            # matmul in it) and one stop on each bank's last matmul.
            for dc in range(NDC):
                nc.tensor.matmul(
                    rt_ps[:, dc, :],
                    lhsT=value[:, sc, dc * 128:(dc + 1) * 128],
                    rhs=atm,
                    start=(sc == 0 and dc in (0, 2)),
                    stop=(sc == NSC - 1 and dc in (1, 3)),
                )

        # ---- copy retrievedT to SBUF + squares ----
        for dc in range(NDC):
            nc.vector.tensor_copy(rT_sb[:, dc, :], rt_ps[:, dc, :])
        # squares on DVE (keeps ACT free so the sqrt table load runs early)
        for dc in range(NDC):
            nc.vector.tensor_mul(out=rsq[:, dc, :], in0=rT_sb[:, dc, :],
                                 in1=rT_sb[:, dc, :])

        # ---- row stats: sums / sumsq in [t,1]; row-sums in [1,t] ----
        # All 16 column-wise stat matmuls share one PSUM bank -> one
        # accumulation group: single start (clears the bank), single stop.
        sums_ps = ps_stat.tile([128, 4], FP32)  # cols: [sum s0, sum s1, sq s0, sq s1]
        row_ps = ps_stat.tile([1, 256], FP32, tag="row")
        first = True
        for st in range(2):
            for src, col in ((rT_sb, st), (rsq, 2 + st)):
                for dc in range(NDC):
                    nc.tensor.matmul(
                        sums_ps[:, col:col + 1],
                        lhsT=src[:, dc, st * 128:(st + 1) * 128],
                        rhs=ones,
                        start=first,
                        stop=(st == 1 and col == 3 and dc == 3),
                    )
                    first = False
        for dc in range(NDC):
            nc.tensor.matmul(
                row_ps,
                lhsT=ones,
                rhs=rT_sb[:, dc, :],
                start=(dc == 0),
                stop=(dc == 3),
            )

        # negmu_row = -(row sums)/D  (bf16, feeds the rank-1 mean-fold matmul)
        negmu = small.tile([1, 256], BF16)
        nc.vector.tensor_scalar_mul(out=negmu, in0=row_ps, scalar1=-1.0 / D)

        # per-strip scale_t = 1/sqrt(var + eps*norm^2)
        import concourse.mybir as _mb
        ALU = _mb.AluOpType
        mu = small.tile([128, 2], FP32)
        musq = small.tile([128, 2], FP32)
        var = small.tile([128, 2], FP32)
        scl = small.tile([128, 2], FP32)
        for st in range(2):
            nc.vector.tensor_scalar_mul(out=mu[:, st:st + 1],
                                        in0=sums_ps[:, st:st + 1], scalar1=1.0 / D)
            nc.vector.tensor_mul(out=musq[:, st:st + 1],
                                 in0=mu[:, st:st + 1], in1=mu[:, st:st + 1])
            # var = sumsq/D - mu^2
            nc.vector.scalar_tensor_tensor(
                out=var[:, st:st + 1],
                in0=sums_ps[:, 2 + st:3 + st],
                scalar=1.0 / D,
                in1=musq[:, st:st + 1],
                op0=ALU.mult,
                op1=ALU.subtract,
            )
        for st in range(2):
            nc.scalar.activation(out=scl[:, st:st + 1], in_=var[:, st:st + 1],
                                 func=AF.Sqrt, bias=epsn2[:, st:st + 1], scale=1.0)
            nc.vector.reciprocal(out=scl[:, st:st + 1], in_=scl[:, st:st + 1])

        # ---- output: out = scale * (rT^T @ Wg - mu*cw) + xplus ----
        for st in range(2):
            ps = ps_big.tile([128, 512], FP32, tag="mlp")
            for dc in range(NDC):
                nc.tensor.matmul(
                    ps,
                    lhsT=rT_sb[:, dc, st * 128:(st + 1) * 128],
                    rhs=wg[:, dc, :],
                    start=(dc == 0),
                    stop=False,
                )
            nc.tensor.matmul(
                ps,
                lhsT=negmu[:, st * 128:(st + 1) * 128],
                rhs=cw,
                start=False,
                stop=True,
            )
            nc.vector.scalar_tensor_tensor(
                out=out_sb[:, st, :],
                in0=ps,
                scalar=scl[:, st:st + 1],
                in1=xplus[:, st, :],
                op0=ALU.mult,
                op1=ALU.add,
            )
            nc.sync.dma_start(out=d_out[st], in_=out_sb[:, st, :])

    return nc


def _host_prepare(inputs):
    """Build the 8 per-core input maps (all host-side numpy)."""
    import ml_dtypes

    bf16 = ml_dtypes.bfloat16
    f32 = np.float32

    x = np.asarray(inputs["x"], f32)
    Wk1 = np.asarray(inputs["Wk1"], f32)
    bk1 = np.asarray(inputs["bk1"], f32)
    Wk2 = np.asarray(inputs["Wk2"], f32)
    bk2 = np.asarray(inputs["bk2"], f32)
    Wq1 = np.asarray(inputs["Wq1"], f32)
    bq1 = np.asarray(inputs["bq1"], f32)
    Wq2 = np.asarray(inputs["Wq2"], f32)
    bq2 = np.asarray(inputs["bq2"], f32)
    Wv = np.asarray(inputs["Wv"], f32)
    bv = np.asarray(inputs["bv"], f32)
    ln_g = np.asarray(inputs["ln_g"], f32)
    ln_b = np.asarray(inputs["ln_b"], f32)
    Wo = np.asarray(inputs["Wo"], f32)
    bo = np.asarray(inputs["bo"], f32)

    Wg = (ln_g[:, None] * Wo).astype(bf16)
    cw = Wg.astype(f32).sum(axis=0).astype(bf16).reshape(1, D)
    out_bias = (ln_b @ Wo + bo).astype(f32)

    def pack(w, dt=bf16):  # [D_in, F] -> [128, 4, F]: SBUF tile layout
        return np.ascontiguousarray(
            w.reshape(4, 128, -1).transpose(1, 0, 2).astype(dt))

    wk2d = np.concatenate([Wk2, Wk2], axis=1)  # [512, 128]
    wq2d = np.concatenate([Wq2, Wq2], axis=1)

    base = {
        "wk1": pack(Wk1),
        "wq1": pack(Wq1),
        "wv": pack(Wv),
        "wg": pack(Wg.astype(f32)),
        "wk2d": pack(wk2d),
        "wq2d": pack(wq2d),
        "bk1": np.ascontiguousarray(bk1.reshape(4, 128).T.astype(f32)),
        "bq1": np.ascontiguousarray(bq1.reshape(4, 128).T.astype(f32)),
        "bk2d": np.concatenate([bk2, bk2]).reshape(128, 1).astype(f32),
        "bq2d": np.concatenate([bq2, bq2]).reshape(128, 1).astype(f32),
        "bv": bv.reshape(1, D).astype(f32),
        "cw": cw,
    }

    in_maps = []
    for c in range(NCORES):
        b, i = divmod(c, 4)
        t0, t1 = i * 128, (7 - i) * 128
        xb = x[b]  # [L, D]
        xTb = np.ascontiguousarray(xb.T)  # [D, L]
        xTp = pack(xTb)  # [128, 4, L]
        qx = np.concatenate([xTb[:, t0:t0 + 128], xTb[:, t1:t1 + 128]], axis=1)
        tglob = np.concatenate([np.arange(t0, t0 + 128), np.arange(t1, t1 + 128)])
        srange = np.arange(L)
        mask = (srange[:, None] <= tglob[None, :]).astype(bf16)  # [L, 256]
        xplus = np.stack([xb[t0:t0 + 128], xb[t1:t1 + 128]]) + out_bias
        epsn2 = (EPS * K * (tglob.astype(f32) + 1.0)).reshape(2, 128)
        m = dict(base)
        m["xTa"] = np.ascontiguousarray(xTp[:, :, 0:512])
        m["xTb"] = np.ascontiguousarray(xTp[:, :, 512:1024])
        m["qxT"] = pack(qx)
        m["mask"] = np.ascontiguousarray(
            mask.reshape(NSC, 128, 256).transpose(1, 0, 2))
        m["xplus"] = np.ascontiguousarray(
            xplus.transpose(1, 0, 2).astype(f32))
        m["epsn2"] = np.ascontiguousarray(epsn2.T.astype(f32))
        in_maps.append(m)
    return in_maps


def run(inputs, trace=False):
    from concourse.bass_utils import run_bass_kernel_spmd

    if "nc" not in _CACHE:
        nc = _build_program()
        nc.finalize()
        _CACHE["nc"] = nc
    nc = _CACHE["nc"]
    in_maps = _host_prepare(inputs)
    res = run_bass_kernel_spmd(nc, in_maps, list(range(NCORES)), trace=trace)
    out = np.empty((B, L, D), np.float32)
    for c in range(NCORES):
        b, i = divmod(c, 4)
        oc = np.asarray(res.results[c]["out"], np.float32)
        out[b, i * 128:(i + 1) * 128] = oc[0]
        out[b, (7 - i) * 128:(8 - i) * 128] = oc[1]
    return out, res


def kernel(**inputs):
    out, _ = run(inputs, trace=False)
    return out
